# revision 59
# baseline (speedup 1.0000x reference)
"""Causal self-attention (B=2, T=2048, D=1024, H=16) on 8 trn2 cores.

Sharding: tensor-parallel over heads x data-parallel over batch.
Core c handles batch b = c // 4, head group g = c % 4 (heads 4g..4g+3).
Host pre-slices/pre-transposes weight+activation shards (cast to bf16,
laid out so every DMA moves long contiguous runs); each core returns a
partial y (its heads' contribution); host sums groups of 4.

Kernel structure (all matmuls bf16, psum f32):
  Attention runs as a chunk-granular pipeline per (t-tile, head pair):
  QK(ss) -> exp(ss) on scalar -> PV(ss-2), with independent "filler"
  matmul groups (qkv projection of a later tile, output projection of
  an earlier tile) interleaved between chunks so the PE queue never
  drains while scalar works on exp.
  PSUM banks: 4 for the QK/exp ping-pong, 2 for the PV accumulators
  (ones-trick rows 64:128 accumulate the softmax denominator L), 2 for
  projection/outproj groups (1 bank each, drained by DVE casts).
  DMAs: only the two hardware descriptor-gen rings (sync / scalar) are
  used; inputs are streamed in first-needed-first order with the first
  attention phase starting before the projection of tile 0 finishes.
  Causal masking: s-chunks beyond the diagonal are skipped entirely;
  diagonal chunks stream only the t >= s columns plus an affine_select
  for the 128-wide triangle.
  The last tile's outproj is split by head pair: pr=0 runs as fillers
  inside the final attention phase (partial written to `ypart`, host
  adds), so the tail after the last exp is only the pr=1 half.
"""

import os
import sys

for _p in ("/opt/trn_rl_repo", "/root/.axon_site/_ro/trn_rl_repo"):
    if os.path.isdir(_p) and _p not in sys.path:
        sys.path.insert(0, _p)

import ml_dtypes
import numpy as np

import concourse.bass as bass
import concourse.mybir as mybir
import concourse.tile as tile
from concourse import bacc
from concourse.bass_utils import run_bass_kernel_spmd

F32 = mybir.dt.float32
BF16 = mybir.dt.bfloat16
U16 = mybir.dt.uint16

B, T, C = 2, 2048, 1024
NHEAD_TOT = 16
DH = 64
NCORES = 8
NH = 4          # heads per core
NPAIR = 2       # head pairs per core
CK = C // 128   # contraction chunks (8)
TT = 512        # t-tile width
NTT = T // TT   # 4
NFG = 6         # 128-col groups of wqkv cols: [q01 q23 k01 k23 v01 v23]
FQK = 2 * NH * DH  # 512 cols of qkv^T for q+k
FV = NH * DH       # 256 cols for v
ONE_BF16 = 0x3F80


def build_nc():
    nc = bacc.Bacc("TRN2", target_bir_lowering=False, debug=False)

    # host-prepped layouts: partition-major, fully contiguous rows
    xTp = nc.dram_tensor("xTp", [128, NTT, CK, TT], BF16, kind="ExternalInput")
    wqkvp = nc.dram_tensor("wqkvp", [128, NFG, CK, 128], BF16, kind="ExternalInput")
    woutp = nc.dram_tensor("woutp", [128, NPAIR, C], BF16, kind="ExternalInput")
    yp_d = nc.dram_tensor("yp_d", [128, NTT, 4, C], BF16, kind="ExternalOutput")
    ypart = nc.dram_tensor("ypart", [128, 4, C], BF16, kind="ExternalOutput")

    EXP = mybir.ActivationFunctionType.Exp

    with tile.TileContext(nc) as tc:
        with (
            tc.tile_pool(name="const", bufs=1) as const,
            tc.tile_pool(name="ptp", bufs=4) as ptp,
            tc.tile_pool(name="rcp", bufs=4) as rcp,
            tc.tile_pool(name="yp", bufs=2) as yp,
            tc.tile_pool(name="psS", bufs=2, space="PSUM") as psS,
            tc.tile_pool(name="psV", bufs=2, space="PSUM") as psV,
            tc.tile_pool(name="psF", bufs=2, space="PSUM") as psF,
        ):
            # ---- persistent SBUF ----
            xT_sb = const.tile([128, NTT, CK, TT], BF16)      # x^T, tile/ci-major
            wqkvT_sb = const.tile([128, NFG, CK, 128], BF16)  # fg-major qkv weights
            woutT_sb = const.tile([128, NPAIR, C], BF16)      # W_out^T rows per head pair
            qk_t = [const.tile([128, 4, TT], BF16, name=f"qk_t{i}") for i in range(NTT)]  # [qp0|qp1|kp0|kp1]
            v_t = [const.tile([128, 4, NH, DH + 64], BF16, name=f"v_t{i}") for i in range(NTT)]  # V + 64 ones cols
            o_t = [[const.tile([128, TT], BF16, name=f"o_t{i}_{p}") for p in range(NPAIR)]
                   for i in range(NTT)]  # normalized O^T, per head pair

            for tt in range(NTT):
                nc.vector.memset(v_t[tt][:, :, :, DH:DH + 64].bitcast(U16), ONE_BF16)

            # ---- input DMAs: 2 hw rings, first-needed-first ----
            # need order for phase A(0) = [f0, f2, v0, v1, f1, f3]:
            # fg0, x(ci01), x(ci23), fg2, x(ci45), fg45(v), x(ci67), fg1, fg3
            ring = [nc.sync, nc.scalar]

            def dma2(dst, src, r):
                ring[r % 2].dma_start(dst, src)

            # ring 0 (sync):   fg0 | x0.ci23 | x0.ci67 | fg45(v) | x1b | x2a | x3a
            # ring 1 (scalar): x0.ci01 | x0.ci45 | fg2 | fg1 | fg3 | x1a | x2b | x3b
            # gpsimd (slow sw ring): only W_out, which is needed last
            dma2(wqkvT_sb[:, 0], wqkvp[:, 0], 0)
            dma2(xT_sb[:, 0, 0:2], xTp[:, 0, 0:2], 1)
            dma2(xT_sb[:, 0, 2:4], xTp[:, 0, 2:4], 0)
            dma2(xT_sb[:, 0, 4:6], xTp[:, 0, 4:6], 1)
            dma2(xT_sb[:, 0, 6:8], xTp[:, 0, 6:8], 0)
            dma2(wqkvT_sb[:, 2], wqkvp[:, 2], 1)
            dma2(wqkvT_sb[:, 4:6], wqkvp[:, 4:6], 0)
            dma2(wqkvT_sb[:, 1], wqkvp[:, 1], 1)
            dma2(wqkvT_sb[:, 3], wqkvp[:, 3], 1)
            dma2(xT_sb[:, 1, 0:4], xTp[:, 1, 0:4], 1)
            dma2(xT_sb[:, 1, 4:8], xTp[:, 1, 4:8], 0)
            dma2(xT_sb[:, 2, 0:4], xTp[:, 2, 0:4], 1)
            dma2(xT_sb[:, 2, 4:8], xTp[:, 2, 4:8], 0)
            nc.gpsimd.dma_start(woutT_sb[:, :, :], woutp[:, :, :])
            dma2(xT_sb[:, 3, 0:4], xTp[:, 3, 0:4], 1)
            dma2(xT_sb[:, 3, 4:8], xTp[:, 3, 4:8], 0)

            # ---------- filler groups: qkv projection + output projection ----
            def emit_proj_qk(tt, f):
                """One [128,512] psum group: qkv-proj of f-th 128 cols of qk."""
                ps = psF.tile([128, TT], F32, tag="psf", name=f"qk{tt}_{f}")
                for ci in range(CK):
                    nc.tensor.matmul(
                        ps,
                        wqkvT_sb[:, f, ci, :],
                        xT_sb[:, tt, ci, :],
                        start=(ci == 0), stop=(ci == CK - 1),
                    )
                nc.vector.tensor_copy(qk_t[tt][:, f, :], ps)

            def emit_proj_v(tt, sp):
                """One [128,2,256] psum group: v projection of 2 s-chunks."""
                ps = psF.tile([128, 2, FV], F32, tag="psf", name=f"v{tt}_{sp}")
                for k in range(2):
                    si = sp * 2 + k
                    for ci in range(CK):
                        nc.tensor.matmul(
                            ps[:, k, :],
                            xT_sb[:, tt, ci, si * 128:(si + 1) * 128],
                            wqkvT_sb[:, 4:6, ci, :],
                            start=(ci == 0), stop=(ci == CK - 1),
                        )
                nc.vector.tensor_copy(
                    v_t[tt][:, sp * 2:sp * 2 + 2, :, 0:DH],
                    ps.rearrange("p k (h d) -> p k h d", h=NH),
                )

            yt_tiles = {}

            def emit_outproj(tt, tq, ot, prs=(0, 1), dst=None, pool=None,
                             cast_eng=None):
                """One [128,512] psum group: y[tt,tq] cols [ot*512:(ot+1)*512].
                `prs` selects which head pairs to accumulate; `dst` overrides
                the output dram tensor (used for the pr-split last tile)."""
                ps = (pool or psF).tile([128, TT], F32, tag="psf" if pool is None else "ps",
                                        name=f"y{tt}_{tq}_{ot}_{prs[0]}")
                for j, pr in enumerate(prs):
                    nc.tensor.matmul(
                        ps,
                        o_t[tt][pr][:, tq * 128:(tq + 1) * 128],
                        woutT_sb[:, pr, ot * TT:(ot + 1) * TT],
                        start=(j == 0), stop=(j == len(prs) - 1),
                    )
                key = (tt, prs)
                if key not in yt_tiles:
                    yt_tiles[key] = [yp.tile([128, 4, 2, TT], BF16, tag="yt",
                                             name=f"yt{tt}_{prs[0]}"), 0]
                ent = yt_tiles[key]
                if cast_eng is nc.scalar:
                    nc.scalar.copy(ent[0][:, tq, ot, :], ps)
                else:
                    nc.vector.tensor_copy(ent[0][:, tq, ot, :], ps)
                ent[1] += 1
                if ent[1] in (4, 8):
                    half = ent[1] // 4 - 1  # 0 or 1
                    target = yp_d[:, tt] if dst is None else dst
                    hs = slice(half * 2, half * 2 + 2)
                    final = tt == NTT - 1
                    src = ent[0][:, hs].rearrange("p a o b -> p (a o b)")
                    tgt = target[:, hs].rearrange("p a b -> p (a b)")
                    if final:  # split across both rings to shorten the tail
                        dma2(tgt[0:64], src[0:64], 0)
                        dma2(tgt[64:128], src[64:128], 1)
                    else:
                        dma2(tgt, src, 0)
                    if ent[1] == 8:
                        del yt_tiles[key]

            # ---------- attention ----------
            def phase_b(tt, pr, pending, fillers):
                """Attention for (t-tile, head pair), depth-2 pipelined QK->PV.
                `pending` (the previous pair's norm emitter) is flushed at
                phase start so the pv psum bufs recycle early. `fillers` is a
                list of (emitter, _) pairs spread evenly over the chunks;
                leftovers carry into the next phase, with one interleaved
                between the two flush PVs (which wait on the last exps)."""
                n_ss = 4 * (tt + 1)
                if pending is not None:
                    pending()
                    pending = None
                pv = [psV.tile([128, TT], F32, tag="pv", name=f"pv{tt}_{pr}_{hi}")
                      for hi in range(2)]

                def emit_pv(pt, t0, ss):
                    for hi in range(2):
                        nc.tensor.matmul(
                            pv[hi][:, t0:TT],
                            v_t[ss // 4][:, ss % 4, pr * 2 + hi, :],
                            pt[:, hi, t0:TT],
                            start=(ss == 0), stop=(ss == n_ss - 1),
                            skip_group_check=True,
                        )

                pipe = []
                n_fill = len(fillers)
                fdone = 0
                for ss in range(n_ss):
                    t0 = max(0, 128 * ss - TT * tt)
                    ps = psS.tile([128, 2, TT], F32, tag="ps", name=f"s{tt}_{pr}_{ss}")
                    for hi in range(2):
                        nc.tensor.matmul(
                            ps[:, hi, t0:TT],
                            qk_t[ss // 4][hi * 64:(hi + 1) * 64, 2 + pr,
                                          (ss % 4) * 128:(ss % 4 + 1) * 128],
                            qk_t[tt][hi * 64:(hi + 1) * 64, pr, t0:TT],
                        )
                    pt = ptp.tile([128, 2, TT], BF16, tag="pt", name=f"pt{tt}_{pr}_{ss}")
                    nc.scalar.activation(pt[:, :, t0:TT], ps[:, :, t0:TT], EXP, scale=0.125)
                    if ss >= 4 * tt:  # diagonal chunk: zero the s > t triangle
                        nc.gpsimd.affine_select(
                            out=pt[:, :, t0:t0 + 128],
                            in_=pt[:, :, t0:t0 + 128],
                            compare_op=mybir.AluOpType.is_ge,
                            fill=0.0,
                            base=0,
                            channel_multiplier=-1,
                            pattern=[[0, 2], [1, 128]],
                        )
                    pipe.append((pt, t0, ss))
                    if len(pipe) > 2:
                        emit_pv(*pipe.pop(0))
                    # spread fillers evenly, leaving ~2 to carry into the next
                    # phase's startup
                    due = ((ss + 1) * n_fill) // (n_ss + 6)
                    while fillers and fdone < due:
                        fillers.pop(0)[0]()
                        fdone += 1
                for item in pipe:
                    emit_pv(*item)
                return pv, fillers

            def norm(tt, pr, pv, final=False):
                """o = pv[0:64] / L; pv[64:128] all hold L (64 ones cols in
                v), so one 64-wide reciprocal IS the partition broadcast."""
                for hi in range(2):
                    lc = rcp.tile([64, TT], F32, tag="lcr", name=f"lc{tt}_{pr}_{hi}")
                    # on the final norm scalar is idle: split the L copies
                    if final and hi == 0:
                        nc.scalar.copy(lc, pv[hi][64:128, :])
                    else:
                        nc.vector.tensor_copy(lc, pv[hi][64:128, :])
                    rc = rcp.tile([64, TT], F32, tag="rcr", name=f"rc{tt}_{pr}_{hi}")
                    nc.vector.reciprocal_approx_fast(out=rc, in_=lc)
                    nc.vector.tensor_mul(
                        o_t[tt][pr][hi * 64:(hi + 1) * 64, :],
                        pv[hi][0:DH, :],
                        rc,
                    )

            # ---------- schedule ----------
            # fillers are (emitter, tile) pairs; `tile` marks projection
            # groups that MUST be emitted before phase (tile, 0) reads their
            # qk_t/v_t output (emission order is the dependency order)
            def proj_groups(tt):
                fs = [(lambda f=f: emit_proj_qk(tt, f), tt) for f in (0, 2)]
                fs += [(lambda sp=sp: emit_proj_v(tt, sp), tt) for sp in range(2)]
                fs += [(lambda f=f: emit_proj_qk(tt, f), tt) for f in (1, 3)]
                return fs

            def outproj_groups(tt, prs=(0, 1), dst=None):
                return [(lambda tq=tq, ot=ot: emit_outproj(tt, tq, ot, prs, dst),
                         None)
                        for tq in range(4) for ot in range(2)]

            filler_map = {
                (0, 0): proj_groups(1),
                (0, 1): proj_groups(2),
                (1, 0): proj_groups(3),
                (1, 1): outproj_groups(0),
                (2, 0): outproj_groups(1)[:4],
                (2, 1): outproj_groups(1)[4:],
                (3, 0): outproj_groups(2),
                (3, 1): outproj_groups(3, prs=(0,), dst=ypart),
            }

            # A(0): only the pr=0 slices + v before attention starts; the
            # pr=1 slices (f1, f3) become the first fillers of B(0,0)
            # pre-phase: only the q/k pr=0 slices B(0,0) strictly needs; the
            # v projections and pr=1 slices flow in as ss0-safe fillers
            a0 = proj_groups(0)
            for g, _ in (a0[0], a0[1]):
                g()
            pending = None
            carry = a0[2:]
            for tt in range(NTT):
                for pr in range(NPAIR):
                    if pr == 0:
                        # force any carried projection groups of THIS tile out
                        # before its attention reads them
                        for g, gt in [f for f in carry if f[1] == tt]:
                            g()
                        carry = [f for f in carry if f[1] != tt]
                    fs = carry + filler_map[(tt, pr)]
                    pv_, carry = phase_b(tt, pr, pending, fs)
                    pending = (lambda a=tt, b=pr, c=pv_: norm(a, b, c))
            for g, _ in carry:
                g()
            # final norm + outproj, fused: normalize o_t[3][1] in tq-column
            # slices so each outproj group starts as soon as its slice is
            # ready; alternate psum pools and cast engines to pipeline 4-deep
            tl = NTT - 1
            rcs = []
            for hi in range(2):
                lc = rcp.tile([64, TT], F32, tag="lcr", name=f"lcF_{hi}")
                if hi == 0:
                    nc.scalar.copy(lc, pv_[hi][64:128, :])
                else:
                    nc.vector.tensor_copy(lc, pv_[hi][64:128, :])
                rc = rcp.tile([64, TT], F32, tag="rcr", name=f"rcF_{hi}")
                nc.vector.reciprocal_approx_fast(out=rc, in_=lc)
                rcs.append(rc)
            i = 0
            for tq in range(4):
                cs = slice(tq * 128, (tq + 1) * 128)
                for hi in range(2):
                    nc.vector.tensor_mul(
                        o_t[tl][1][hi * 64:(hi + 1) * 64, cs],
                        pv_[hi][0:DH, cs],
                        rcs[hi][:, cs],
                    )
                for ot in range(2):
                    emit_outproj(tl, tq, ot, prs=(1,),
                                 pool=psS if i % 2 else None,
                                 cast_eng=nc.scalar if i % 2 else None)
                    i += 1

    nc.compile()
    return nc


_NC_CACHE = None


def _get_nc():
    global _NC_CACHE
    if _NC_CACHE is None:
        _NC_CACHE = build_nc()
    return _NC_CACHE


def make_in_maps(x, W_qkv, W_out):
    bf = ml_dtypes.bfloat16
    x = np.asarray(x, dtype=np.float32)
    W_qkv = np.asarray(W_qkv, dtype=np.float32)
    W_out = np.asarray(W_out, dtype=np.float32)
    # xTp[p, tt, ci, t] = x[b, tt*TT + t, ci*128 + p]
    xTp = [np.ascontiguousarray(
        x[b].reshape(NTT, TT, CK, 128).transpose(3, 0, 2, 1).astype(bf))
        for b in range(B)]
    in_maps = []
    for c in range(NCORES):
        b, g = c // 4, c % 4
        rq = W_qkv[g * 256:(g + 1) * 256]            # q rows, heads 4g..4g+3
        rk = W_qkv[C + g * 256:C + (g + 1) * 256]    # k rows
        rv = W_qkv[2 * C + g * 256:2 * C + (g + 1) * 256]  # v rows
        # wqkvp[p, fg, ci, fi] = W_rows[fg*128 + fi, ci*128 + p]
        wrows = np.concatenate([rq, rk, rv], axis=0)  # [768, C]
        wqkvp = np.ascontiguousarray(
            wrows.T.reshape(CK, 128, NFG, 128).transpose(1, 2, 0, 3).astype(bf))
        # woutp[p, pr, c] = W_out[c, g*256 + pr*128 + p]
        wout_g = W_out[:, g * 256:(g + 1) * 256].T   # [256, C]
        woutp = np.ascontiguousarray(
            wout_g.reshape(NPAIR, 128, C).transpose(1, 0, 2).astype(bf))
        in_maps.append({"xTp": xTp[b], "wqkvp": wqkvp, "woutp": woutp})
    return in_maps


def kernel(x, W_qkv, W_out):
    nc = _get_nc()
    in_maps = make_in_maps(x, W_qkv, W_out)
    res = run_bass_kernel_spmd(nc, in_maps, core_ids=list(range(NCORES)))
    kernel.last_results = res
    y = np.zeros((B, T, C), dtype=np.float32)
    for c in range(NCORES):
        # y_core[tt*512 + tq*128 + p, :] = yp_d[p, tt, tq, :]
        ypd = res.results[c]["yp_d"].astype(np.float32)
        yc = ypd.transpose(1, 2, 0, 3).reshape(T, C)
        yprt = res.results[c]["ypart"].astype(np.float32)
        yc[(NTT - 1) * TT:] += yprt.transpose(1, 0, 2).reshape(TT, C)
        y[c // 4] += yc
    return y
